# revision 28
# baseline (speedup 1.0000x reference)
"""Bass/Trainium2 kernel for nn_FourierBlock (rfft -> per-mode complex einsum -> irfft).

Math (per head h):
  X[m_ri, (b,i)]   = FB.T @ xT          forward DFT, only 64 modes needed
  Mst[i_ri, (b,m)] = per-b transposes   (plain PE matmuls vs identity)
  O[o_ri, (m,b)]   = S_m.T @ Mst_m      per-mode stacked-complex einsum
  P_k[m_ri,(b2,o)] = per-b transposes   (plain PE matmuls vs identity)
  Y[(b2,o), l]     = P_k.T @ G          inverse DFT (64 modes -> 1024 samples)

Sharding: one head per NeuronCore (8 heads, 8 cores). Weights per head are
private to the core; no communication.

Perf notes (measured on trn2 via perfetto/NTFF traces):
- Transposes are emitted as plain matmuls (lhsT=data, rhs=identity slice)
  grouped by PE quadrant; they pipeline at ~20-50ns/instr vs ~200ns for the
  is_transpose path. Math is exact either way.
- Warmup matmuls keep PE busy while input DMA streams, holding the PE p-state
  clock up so the first fwd-DFT matmuls run ~375ns instead of ~600ns.
- All input DMA rides one HWDGE queue (sync) in priority order: fb head chunk
  (unblocks the first fwd matmul), xq0, rest of fb, xq (small leading chunks
  because the DMA rate ramps ~160->430 B/ns over the first ~6us), then s, g.
  Splitting input across both queues just splits the shared ~400 B/ns port.
- Forward DFT accumulates chunk-by-chunk as xq chunks land; einsum modes
  chase the s chunks; per-k inverse DFT output is copied (vector/scalar
  alternating) and DMA'd per k so the output drain overlaps the inverse DFT.
- PSUM: phalf bufs=3 (warmup + fwd halves + stage A/einsum/B), pyh bufs=5
  (inverse-DFT halves, depth ~2.5 k) = 8 banks. pyh<5 stalls the tail.
Dead ends (measured worse or faulted): 1024-col matmul out (neuronxcc
rejects), strided stationary (slow loads), strided PSUM matmul out (device
UNRECOVERABLE), fp8 anywhere (rel-err budget), input on both queues, halving
s via on-device mirror (engine work exceeds the 2.8us DMA saving).
"""

import numpy as np
import ml_dtypes

import concourse.bass as bass
import concourse.mybir as mybir
import concourse.tile as tile
from concourse import bacc
from concourse.bass_utils import run_bass_kernel_spmd
from concourse.masks import make_identity

B, L, H, E, M = 16, 1024, 8, 64, 64
BF = mybir.dt.bfloat16
F32 = mybir.dt.float32
NPBF = ml_dtypes.bfloat16

N_WARMUP = 24


def kernel_body(tc, outs, ins):
    nc = tc.nc
    ys = outs
    xq, fb, s, g = ins

    with (
        tc.tile_pool(name="const", bufs=1) as const,
        tc.tile_pool(name="work", bufs=1) as work,
        tc.tile_pool(name="yout", bufs=8) as yout,
        tc.tile_pool(name="phalf", bufs=3, space="PSUM") as phalf,
        tc.tile_pool(name="pyh", bufs=5, space="PSUM") as pyh,
    ):
        # ---- input DMA: all on the sync queue, priority-ordered ----
        fb_sb = const.tile([128, 8 * 128], BF, tag="fb")
        # head chunk first: the c=0 stationary (32KB) unblocks the first
        # fwd-DFT matmul ~2us before the rest of fb/xq lands.
        nc.sync.dma_start(fb_sb[:, 0:128], fb[:, 0:128])
        xq_sb = const.tile([128, 8 * 1024], BF, tag="xq")
        s_sb = const.tile([128, 64 * 128], BF, tag="s")
        def xq_dma_cols(lo, hi):
            nc.sync.dma_start(xq_sb[:, lo:hi], xq[:, lo:hi])
        def s_dma(c):
            nc.sync.dma_start(
                s_sb[:, c * 2048 : (c + 1) * 2048], s[:, c * 2048 : (c + 1) * 2048]
            )
        # leading chunks are small so the first fwd-DFT matmuls start while
        # the DMA rate is still ramping; trailing chunks are big for rate.
        xq_dma_cols(0, 1024)
        nc.sync.dma_start(fb_sb[:, 128:1024], fb[:, 128:1024])
        for lo, hi in ((1024, 2048), (2048, 3072), (3072, 4096),
                       (4096, 6144), (6144, 8192)):
            xq_dma_cols(lo, hi)
        for c in range(4):
            s_dma(c)
        g_sb = const.tile([128, 1024], BF, tag="g")
        nc.sync.dma_start(g_sb[:], g[:])

        ident = const.tile([128, 128], BF, tag="ident")
        make_identity(nc, ident[:])

        # ---- PE warmup: ramp the clock while input DMA streams ----
        warm = phalf.tile([128, 512], F32, tag="half")
        for _ in range(N_WARMUP):
            nc.tensor.matmul(warm[:, 0:128], ident[:], ident[:], start=True, stop=True)

        # ---- forward DFT: X[m_ri, (b,i)]; accumulate both halves per chunk ----
        Xp0 = phalf.tile([128, 512], F32, tag="half")
        Xp1 = phalf.tile([128, 512], F32, tag="half")
        for c in range(8):
            nc.tensor.matmul(
                Xp0[:],
                fb_sb[:, c * 128 : (c + 1) * 128],
                xq_sb[:, c * 1024 : c * 1024 + 512],
                start=(c == 0),
                stop=(c == 7),
            )
            nc.tensor.matmul(
                Xp1[:],
                fb_sb[:, c * 128 : (c + 1) * 128],
                xq_sb[:, c * 1024 + 512 : (c + 1) * 1024],
                start=(c == 0),
                stop=(c == 7),
            )
        X_sb = work.tile([128, 1024], BF, tag="xsb")
        nc.vector.tensor_copy(X_sb[:, 0:512], Xp0[:])
        nc.scalar.copy(X_sb[:, 512:1024], Xp1[:])

        # ---- stage A transposes: Mst[i_ri, (b,m)]  (free col = b*64 + m) ----
        # plain matmuls: out = lhsT.T @ I, grouped h0-then-h64 per PSUM tile.
        Mst = work.tile([128, 1024], BF, tag="mst")
        for half in (0, 1):
            Mp = phalf.tile([128, 512], F32, tag="half")
            bs = range(half * 8, half * 8 + 8)
            for b in bs:
                cols = slice(b * 64 - half * 512, (b + 1) * 64 - half * 512)
                icols = slice(b * 64, (b + 1) * 64)
                nc.tensor.matmul(
                    Mp[0:64, cols], X_sb[0:64, icols], ident[0:64, 0:64],
                    start=True, stop=True,
                )
            for b in bs:
                cols = slice(b * 64 - half * 512, (b + 1) * 64 - half * 512)
                icols = slice(b * 64, (b + 1) * 64)
                nc.tensor.matmul(
                    Mp[64:128, cols], X_sb[64:128, icols], ident[64:128, 64:128],
                    start=True, stop=True,
                )
            if half == 0:
                nc.vector.tensor_copy(Mst[:, 0:512], Mp[:])
            else:
                nc.scalar.copy(Mst[:, 512:1024], Mp[:])

        # ---- einsum: O[o_ri, (m,b)] -> stored b-major in O_sb ----
        O_sb = work.tile([128, 1024], BF, tag="osb")
        for half in (0, 1):
            Op = phalf.tile([128, 512], F32, tag="half")
            for m in range(half * 32, half * 32 + 32):
                nc.tensor.matmul(
                    Op[:, m * 16 - half * 512 : (m + 1) * 16 - half * 512],
                    s_sb[:, m * 128 : (m + 1) * 128],
                    Mst[:, m : 1024 : 64],
                    start=True,
                    stop=True,
                )
            for q in (0, 1):
                mm_lo = half * 32 + q * 16
                dst = O_sb.rearrange("p (b mm) -> p b mm", b=16)[
                    :, :, mm_lo : mm_lo + 16
                ]
                srcq = Op.rearrange("p (mm b) -> p b mm", b=16)[
                    :, :, q * 16 : (q + 1) * 16
                ]
                if half == 0 or q == 0:
                    nc.scalar.copy(dst, srcq)
                else:
                    nc.vector.tensor_copy(dst, srcq)

        # ---- stage B transposes: lhsT_all[m_ri, (k, j, o)] ----
        lhsT_all = work.tile([128, 1024], BF, tag="lhsT")
        for half in (0, 1):
            Pp = phalf.tile([128, 512], F32, tag="half")
            kjs = [(k, j) for k in range(half * 4, half * 4 + 4) for j in (0, 1)]
            for k, j in kjs:
                b = 2 * k + j
                cols = slice(
                    k * 128 + j * 64 - half * 512,
                    k * 128 + (j + 1) * 64 - half * 512,
                )
                bcols = slice(b * 64, (b + 1) * 64)
                nc.tensor.matmul(
                    Pp[0:64, cols], O_sb[0:64, bcols], ident[0:64, 0:64],
                    start=True, stop=True,
                )
            for k, j in kjs:
                b = 2 * k + j
                cols = slice(
                    k * 128 + j * 64 - half * 512,
                    k * 128 + (j + 1) * 64 - half * 512,
                )
                bcols = slice(b * 64, (b + 1) * 64)
                nc.tensor.matmul(
                    Pp[64:128, cols], O_sb[64:128, bcols], ident[64:128, 64:128],
                    start=True, stop=True,
                )
            if half == 0:
                nc.vector.tensor_copy(lhsT_all[:, 0:512], Pp[:])
            else:
                nc.scalar.copy(lhsT_all[:, 512:1024], Pp[:])

        # ---- inverse DFT per batch-pair; y DMA alternates sync/scalar queues ----
        for k in range(8):
            Yh0 = pyh.tile([128, 512], F32, tag="yh")
            Yh1 = pyh.tile([128, 512], F32, tag="yh")
            lk = lhsT_all[:, k * 128 : (k + 1) * 128]
            nc.tensor.matmul(Yh0[:], lk, g_sb[:, 0:512], start=True, stop=True)
            nc.tensor.matmul(Yh1[:], lk, g_sb[:, 512:1024], start=True, stop=True)
            y_sb = yout.tile([128, 1024], BF, tag="ysb")
            if k % 2 == 1:
                nc.vector.tensor_copy(y_sb[:, 0:512], Yh0[:])
                nc.vector.tensor_copy(y_sb[:, 512:1024], Yh1[:])
            else:
                nc.scalar.copy(y_sb[:, 0:512], Yh0[:])
                nc.scalar.copy(y_sb[:, 512:1024], Yh1[:])
            nc.sync.dma_start(ys[k][:], y_sb[:])


def build_nc():
    nc = bacc.Bacc(
        "TRN2", target_bir_lowering=False, debug=False, num_devices=8
    )
    xq = nc.dram_tensor("xq", [128, 8 * 1024], BF, kind="ExternalInput").ap()
    fb = nc.dram_tensor("fb", [128, 8 * 128], BF, kind="ExternalInput").ap()
    s = nc.dram_tensor("s", [128, 64 * 128], BF, kind="ExternalInput").ap()
    g = nc.dram_tensor("g", [128, 1024], BF, kind="ExternalInput").ap()
    ys = [
        nc.dram_tensor(f"y{k}", [128, 1024], BF, kind="ExternalOutput").ap()
        for k in range(8)
    ]
    with tile.TileContext(nc) as tc:
        kernel_body(tc, ys, [xq, fb, s, g])
    nc.compile()
    return nc


def host_basis():
    l = np.arange(L, dtype=np.float64)[:, None]
    m = np.arange(M, dtype=np.float64)[None, :]
    ang = 2 * np.pi * l * m / L
    FB = np.concatenate([np.cos(ang), -np.sin(ang)], axis=1)  # [L, 128]
    c = np.full(M, 2.0)
    c[0] = 1.0
    GC = c[:, None] * np.cos(ang.T) / L
    GS = -c[:, None] * np.sin(ang.T) / L
    G = np.concatenate([GC, GS], axis=0)  # [128, L]
    # chunk-major layout for direct [128, ...] DMA
    fb_host = np.ascontiguousarray(
        FB.reshape(8, 128, 128).transpose(1, 0, 2).reshape(128, 1024)
    ).astype(NPBF)
    g_host = np.ascontiguousarray(G).astype(NPBF)
    return fb_host, g_host


def host_inputs(q, w_real, w_imag):
    fb_host, g_host = host_basis()
    in_maps = []
    for h in range(H):
        x = q[:, :, h, :]  # [B, L, E]
        xT = np.transpose(x, (1, 0, 2)).reshape(L, B * E)  # [L, (b,i)] b-major
        xq_host = np.ascontiguousarray(
            xT.reshape(8, 128, B * E).transpose(1, 0, 2).reshape(128, 8 * 1024)
        ).astype(NPBF)
        # einsum stationaries: S_m = [[Wr, Wi], [-Wi, Wr]]  (rows i_ri, cols o_ri)
        Wr = w_real[h].astype(np.float32)  # [i, o, m]
        Wi = w_imag[h].astype(np.float32)
        Sm = np.empty((M, 128, 128), dtype=np.float32)
        Sm[:, 0:64, 0:64] = Wr.transpose(2, 0, 1)
        Sm[:, 0:64, 64:128] = Wi.transpose(2, 0, 1)
        Sm[:, 64:128, 0:64] = -Wi.transpose(2, 0, 1)
        Sm[:, 64:128, 64:128] = Wr.transpose(2, 0, 1)
        s_host = np.ascontiguousarray(Sm.transpose(1, 0, 2).reshape(128, 8192)).astype(
            NPBF
        )
        in_maps.append({"xq": xq_host, "fb": fb_host, "s": s_host, "g": g_host})
    return in_maps


def assemble(results):
    out = np.empty((B, H, E, L), dtype=np.float32)
    for h in range(H):
        yh = np.stack(
            [results[h][f"y{k}"].astype(np.float32) for k in range(8)]
        )  # [k, 128, L]
        out[:, h, :, :] = yh.reshape(B, E, L)  # [(k,j)=b, o, l]
    return out


_NC_CACHE = {}


def run(q, w_real, w_imag, **kwargs):
    if "nc" not in _NC_CACHE:
        _NC_CACHE["nc"] = build_nc()
    nc = _NC_CACHE["nc"]
    in_maps = host_inputs(
        np.asarray(q, dtype=np.float32),
        np.asarray(w_real, dtype=np.float32),
        np.asarray(w_imag, dtype=np.float32),
    )
    res = run_bass_kernel_spmd(nc, in_maps, core_ids=list(range(H)), **kwargs)
    return assemble(res.results), res


def kernel(q, w_real, w_imag):
    return run(q, w_real, w_imag)[0]


# revision 29
# speedup vs baseline: 1.0052x; 1.0052x over previous
"""Bass/Trainium2 kernel for nn_FourierBlock (rfft -> per-mode complex einsum -> irfft).

Math (per head h):
  X[m_ri, (b,i)]   = FB.T @ xT          forward DFT, only 64 modes needed
  Mst[i_ri, (b,m)] = per-b transposes   (plain PE matmuls vs identity)
  O[o_ri, (m,b)]   = S_m.T @ Mst_m      per-mode stacked-complex einsum
  P_k[m_ri,(b2,o)] = per-b transposes   (plain PE matmuls vs identity)
  Y[(b2,o), l]     = P_k.T @ G          inverse DFT (64 modes -> 1024 samples)

Sharding: one head per NeuronCore (8 heads, 8 cores). Weights per head are
private to the core; no communication.

Perf notes (measured on trn2 via perfetto/NTFF traces):
- Transposes are emitted as plain matmuls (lhsT=data, rhs=identity slice)
  grouped by PE quadrant; they pipeline at ~20-50ns/instr vs ~200ns for the
  is_transpose path. Math is exact either way.
- Warmup matmuls keep PE busy while input DMA streams, holding the PE p-state
  clock up so the first fwd-DFT matmuls run ~375ns instead of ~600ns.
- All input DMA rides one HWDGE queue (sync) in priority order: fb head chunk
  (unblocks the first fwd matmul), xq0, rest of fb, xq (small leading chunks
  because the DMA rate ramps ~160->430 B/ns over the first ~6us), then s, g.
  Splitting input across both queues just splits the shared ~400 B/ns port.
- Forward DFT accumulates chunk-by-chunk as xq chunks land; einsum modes
  chase the s chunks; per-k inverse DFT output is copied (vector/scalar
  alternating) and DMA'd per k so the output drain overlaps the inverse DFT.
- PSUM: phalf bufs=3 (warmup + fwd halves + stage A/einsum/B), pyh bufs=5
  (inverse-DFT halves, depth ~2.5 k) = 8 banks. pyh<5 stalls the tail.
Dead ends (measured worse or faulted): 1024-col matmul out (neuronxcc
rejects), strided stationary (slow loads), strided PSUM matmul out (device
UNRECOVERABLE), fp8 anywhere (rel-err budget), input on both queues, halving
s via on-device mirror (engine work exceeds the 2.8us DMA saving).
"""

import numpy as np
import ml_dtypes

import concourse.bass as bass
import concourse.mybir as mybir
import concourse.tile as tile
from concourse import bacc
from concourse.bass_utils import run_bass_kernel_spmd
from concourse.masks import make_identity

B, L, H, E, M = 16, 1024, 8, 64, 64
BF = mybir.dt.bfloat16
F32 = mybir.dt.float32
NPBF = ml_dtypes.bfloat16

N_WARMUP = 24


def kernel_body(tc, outs, ins):
    nc = tc.nc
    ys = outs
    xq, fb, s, g = ins

    with (
        tc.tile_pool(name="const", bufs=1) as const,
        tc.tile_pool(name="work", bufs=1) as work,
        tc.tile_pool(name="yout", bufs=8) as yout,
        tc.tile_pool(name="phalf", bufs=3, space="PSUM") as phalf,
        tc.tile_pool(name="pyh", bufs=5, space="PSUM") as pyh,
    ):
        # ---- input DMA: all on the sync queue, priority-ordered ----
        fb_sb = const.tile([128, 8 * 128], BF, tag="fb")
        # head chunk first: the c=0 stationary (32KB) unblocks the first
        # fwd-DFT matmul ~2us before the rest of fb/xq lands.
        nc.sync.dma_start(fb_sb[:, 0:128], fb[:, 0:128])
        xq_sb = const.tile([128, 8 * 1024], BF, tag="xq")
        s_sb = const.tile([128, 64 * 128], BF, tag="s")
        def xq_dma_cols(lo, hi):
            nc.sync.dma_start(xq_sb[:, lo:hi], xq[:, lo:hi])
        def s_dma(c):
            nc.sync.dma_start(
                s_sb[:, c * 2048 : (c + 1) * 2048], s[:, c * 2048 : (c + 1) * 2048]
            )
        # leading chunks are small so the first fwd-DFT matmuls start while
        # the DMA rate is still ramping; trailing chunks are big for rate.
        xq_dma_cols(0, 1024)
        nc.sync.dma_start(fb_sb[:, 128:1024], fb[:, 128:1024])
        for lo, hi in ((1024, 2048), (2048, 3072), (3072, 4096),
                       (4096, 6144), (6144, 8192)):
            xq_dma_cols(lo, hi)
        for c in range(4):
            s_dma(c)
        g_sb = const.tile([128, 1024], BF, tag="g")

        ident = const.tile([128, 128], BF, tag="ident")
        make_identity(nc, ident[:])

        # ---- PE warmup: ramp the clock while input DMA streams ----
        warm = phalf.tile([128, 512], F32, tag="half")
        for _ in range(N_WARMUP):
            nc.tensor.matmul(warm[:, 0:128], ident[:], ident[:], start=True, stop=True)

        # ---- forward DFT: X[m_ri, (b,i)]; accumulate both halves per chunk ----
        Xp0 = phalf.tile([128, 512], F32, tag="half")
        Xp1 = phalf.tile([128, 512], F32, tag="half")
        for c in range(8):
            nc.tensor.matmul(
                Xp0[:],
                fb_sb[:, c * 128 : (c + 1) * 128],
                xq_sb[:, c * 1024 : c * 1024 + 512],
                start=(c == 0),
                stop=(c == 7),
            )
            nc.tensor.matmul(
                Xp1[:],
                fb_sb[:, c * 128 : (c + 1) * 128],
                xq_sb[:, c * 1024 + 512 : (c + 1) * 1024],
                start=(c == 0),
                stop=(c == 7),
            )
        X_sb = work.tile([128, 1024], BF, tag="xsb")
        nc.vector.tensor_copy(X_sb[:, 0:512], Xp0[:])
        nc.scalar.copy(X_sb[:, 512:1024], Xp1[:])
        # g rides the scalar queue here (late in scalar program order): the
        # sync queue is still busy with s, and this warms q10 for the odd-k
        # output DMAs below.
        nc.scalar.dma_start(g_sb[:], g[:])

        # ---- stage A transposes: Mst[i_ri, (b,m)]  (free col = b*64 + m) ----
        # plain matmuls: out = lhsT.T @ I, grouped h0-then-h64 per PSUM tile.
        Mst = work.tile([128, 1024], BF, tag="mst")
        for half in (0, 1):
            Mp = phalf.tile([128, 512], F32, tag="half")
            bs = range(half * 8, half * 8 + 8)
            for b in bs:
                cols = slice(b * 64 - half * 512, (b + 1) * 64 - half * 512)
                icols = slice(b * 64, (b + 1) * 64)
                nc.tensor.matmul(
                    Mp[0:64, cols], X_sb[0:64, icols], ident[0:64, 0:64],
                    start=True, stop=True,
                )
            for b in bs:
                cols = slice(b * 64 - half * 512, (b + 1) * 64 - half * 512)
                icols = slice(b * 64, (b + 1) * 64)
                nc.tensor.matmul(
                    Mp[64:128, cols], X_sb[64:128, icols], ident[64:128, 64:128],
                    start=True, stop=True,
                )
            if half == 0:
                nc.vector.tensor_copy(Mst[:, 0:512], Mp[:])
            else:
                nc.scalar.copy(Mst[:, 512:1024], Mp[:])

        # ---- einsum: O[o_ri, (m,b)] -> stored b-major in O_sb ----
        O_sb = work.tile([128, 1024], BF, tag="osb")
        for half in (0, 1):
            Op = phalf.tile([128, 512], F32, tag="half")
            for m in range(half * 32, half * 32 + 32):
                nc.tensor.matmul(
                    Op[:, m * 16 - half * 512 : (m + 1) * 16 - half * 512],
                    s_sb[:, m * 128 : (m + 1) * 128],
                    Mst[:, m : 1024 : 64],
                    start=True,
                    stop=True,
                )
            for q in (0, 1):
                mm_lo = half * 32 + q * 16
                dst = O_sb.rearrange("p (b mm) -> p b mm", b=16)[
                    :, :, mm_lo : mm_lo + 16
                ]
                srcq = Op.rearrange("p (mm b) -> p b mm", b=16)[
                    :, :, q * 16 : (q + 1) * 16
                ]
                if half == 0 or q == 0:
                    nc.scalar.copy(dst, srcq)
                else:
                    nc.vector.tensor_copy(dst, srcq)

        # ---- stage B transposes: lhsT_all[m_ri, (k, j, o)] ----
        lhsT_all = work.tile([128, 1024], BF, tag="lhsT")
        for half in (0, 1):
            Pp = phalf.tile([128, 512], F32, tag="half")
            kjs = [(k, j) for k in range(half * 4, half * 4 + 4) for j in (0, 1)]
            for k, j in kjs:
                b = 2 * k + j
                cols = slice(
                    k * 128 + j * 64 - half * 512,
                    k * 128 + (j + 1) * 64 - half * 512,
                )
                bcols = slice(b * 64, (b + 1) * 64)
                nc.tensor.matmul(
                    Pp[0:64, cols], O_sb[0:64, bcols], ident[0:64, 0:64],
                    start=True, stop=True,
                )
            for k, j in kjs:
                b = 2 * k + j
                cols = slice(
                    k * 128 + j * 64 - half * 512,
                    k * 128 + (j + 1) * 64 - half * 512,
                )
                bcols = slice(b * 64, (b + 1) * 64)
                nc.tensor.matmul(
                    Pp[64:128, cols], O_sb[64:128, bcols], ident[64:128, 64:128],
                    start=True, stop=True,
                )
            if half == 0:
                nc.vector.tensor_copy(lhsT_all[:, 0:512], Pp[:])
            else:
                nc.scalar.copy(lhsT_all[:, 512:1024], Pp[:])

        # ---- inverse DFT per batch-pair; y DMA alternates sync/scalar queues ----
        for k in range(8):
            Yh0 = pyh.tile([128, 512], F32, tag="yh")
            Yh1 = pyh.tile([128, 512], F32, tag="yh")
            lk = lhsT_all[:, k * 128 : (k + 1) * 128]
            nc.tensor.matmul(Yh0[:], lk, g_sb[:, 0:512], start=True, stop=True)
            nc.tensor.matmul(Yh1[:], lk, g_sb[:, 512:1024], start=True, stop=True)
            y_sb = yout.tile([128, 1024], BF, tag="ysb")
            if k % 2 == 1:
                nc.vector.tensor_copy(y_sb[:, 0:512], Yh0[:])
                nc.vector.tensor_copy(y_sb[:, 512:1024], Yh1[:])
            else:
                nc.scalar.copy(y_sb[:, 0:512], Yh0[:])
                nc.scalar.copy(y_sb[:, 512:1024], Yh1[:])
            if k % 2 == 0:
                nc.sync.dma_start(ys[k][:], y_sb[:])
            else:
                nc.scalar.dma_start(ys[k][:], y_sb[:])


def build_nc():
    nc = bacc.Bacc(
        "TRN2", target_bir_lowering=False, debug=False, num_devices=8
    )
    xq = nc.dram_tensor("xq", [128, 8 * 1024], BF, kind="ExternalInput").ap()
    fb = nc.dram_tensor("fb", [128, 8 * 128], BF, kind="ExternalInput").ap()
    s = nc.dram_tensor("s", [128, 64 * 128], BF, kind="ExternalInput").ap()
    g = nc.dram_tensor("g", [128, 1024], BF, kind="ExternalInput").ap()
    ys = [
        nc.dram_tensor(f"y{k}", [128, 1024], BF, kind="ExternalOutput").ap()
        for k in range(8)
    ]
    with tile.TileContext(nc) as tc:
        kernel_body(tc, ys, [xq, fb, s, g])
    nc.compile()
    return nc


def host_basis():
    l = np.arange(L, dtype=np.float64)[:, None]
    m = np.arange(M, dtype=np.float64)[None, :]
    ang = 2 * np.pi * l * m / L
    FB = np.concatenate([np.cos(ang), -np.sin(ang)], axis=1)  # [L, 128]
    c = np.full(M, 2.0)
    c[0] = 1.0
    GC = c[:, None] * np.cos(ang.T) / L
    GS = -c[:, None] * np.sin(ang.T) / L
    G = np.concatenate([GC, GS], axis=0)  # [128, L]
    # chunk-major layout for direct [128, ...] DMA
    fb_host = np.ascontiguousarray(
        FB.reshape(8, 128, 128).transpose(1, 0, 2).reshape(128, 1024)
    ).astype(NPBF)
    g_host = np.ascontiguousarray(G).astype(NPBF)
    return fb_host, g_host


def host_inputs(q, w_real, w_imag):
    fb_host, g_host = host_basis()
    in_maps = []
    for h in range(H):
        x = q[:, :, h, :]  # [B, L, E]
        xT = np.transpose(x, (1, 0, 2)).reshape(L, B * E)  # [L, (b,i)] b-major
        xq_host = np.ascontiguousarray(
            xT.reshape(8, 128, B * E).transpose(1, 0, 2).reshape(128, 8 * 1024)
        ).astype(NPBF)
        # einsum stationaries: S_m = [[Wr, Wi], [-Wi, Wr]]  (rows i_ri, cols o_ri)
        Wr = w_real[h].astype(np.float32)  # [i, o, m]
        Wi = w_imag[h].astype(np.float32)
        Sm = np.empty((M, 128, 128), dtype=np.float32)
        Sm[:, 0:64, 0:64] = Wr.transpose(2, 0, 1)
        Sm[:, 0:64, 64:128] = Wi.transpose(2, 0, 1)
        Sm[:, 64:128, 0:64] = -Wi.transpose(2, 0, 1)
        Sm[:, 64:128, 64:128] = Wr.transpose(2, 0, 1)
        s_host = np.ascontiguousarray(Sm.transpose(1, 0, 2).reshape(128, 8192)).astype(
            NPBF
        )
        in_maps.append({"xq": xq_host, "fb": fb_host, "s": s_host, "g": g_host})
    return in_maps


def assemble(results):
    out = np.empty((B, H, E, L), dtype=np.float32)
    for h in range(H):
        yh = np.stack(
            [results[h][f"y{k}"].astype(np.float32) for k in range(8)]
        )  # [k, 128, L]
        out[:, h, :, :] = yh.reshape(B, E, L)  # [(k,j)=b, o, l]
    return out


_NC_CACHE = {}


def run(q, w_real, w_imag, **kwargs):
    if "nc" not in _NC_CACHE:
        _NC_CACHE["nc"] = build_nc()
    nc = _NC_CACHE["nc"]
    in_maps = host_inputs(
        np.asarray(q, dtype=np.float32),
        np.asarray(w_real, dtype=np.float32),
        np.asarray(w_imag, dtype=np.float32),
    )
    res = run_bass_kernel_spmd(nc, in_maps, core_ids=list(range(H)), **kwargs)
    return assemble(res.results), res


def kernel(q, w_real, w_imag):
    return run(q, w_real, w_imag)[0]


# revision 30
# speedup vs baseline: 1.0550x; 1.0495x over previous
"""Bass/Trainium2 kernel for nn_FourierBlock (rfft -> per-mode complex einsum -> irfft).

Math (per head h):
  X[m_ri, (b,i)]   = FB.T @ xT          forward DFT, only 64 modes needed
  Mst[i_ri, (b,m)] = per-b transposes   (plain PE matmuls vs identity)
  O[o_ri, (m,b)]   = S_m.T @ Mst_m      per-mode stacked-complex einsum
  P_k[m_ri,(b2,o)] = per-b transposes   (plain PE matmuls vs identity)
  Y[(b2,o), l]     = P_k.T @ G          inverse DFT (64 modes -> 1024 samples)

Sharding: one head per NeuronCore (8 heads, 8 cores). Weights per head are
private to the core; no communication.

Perf notes (measured on trn2 via perfetto/NTFF traces):
- Transposes are emitted as plain matmuls (lhsT=data, rhs=identity slice)
  grouped by PE quadrant; they pipeline at ~20-50ns/instr vs ~200ns for the
  is_transpose path. Math is exact either way.
- Warmup matmuls keep PE busy while input DMA streams, holding the PE p-state
  clock up so the first fwd-DFT matmuls run ~375ns instead of ~600ns.
- All input DMA rides one HWDGE queue (sync) in priority order: fb head chunk
  (unblocks the first fwd matmul), xq0, rest of fb, xq (small leading chunks
  because the DMA rate ramps ~160->430 B/ns over the first ~6us), then s, g.
  Splitting input across both queues just splits the shared ~400 B/ns port.
- Forward DFT accumulates chunk-by-chunk as xq chunks land; einsum modes
  chase the s chunks; per-k inverse DFT output is copied (vector/scalar
  alternating) and DMA'd per k so the output drain overlaps the inverse DFT.
- PSUM: phalf bufs=3 (warmup + fwd halves + stage A/einsum/B), pyh bufs=5
  (inverse-DFT halves, depth ~2.5 k) = 8 banks. pyh<5 stalls the tail.
Dead ends (measured worse or faulted): 1024-col matmul out (neuronxcc
rejects), strided stationary (slow loads), strided PSUM matmul out (device
UNRECOVERABLE), fp8 anywhere (rel-err budget), input on both queues, halving
s via on-device mirror (engine work exceeds the 2.8us DMA saving).
"""

import numpy as np
import ml_dtypes

import concourse.bass as bass
import concourse.mybir as mybir
import concourse.tile as tile
from concourse import bacc
from concourse.bass_utils import run_bass_kernel_spmd
from concourse.masks import make_identity

B, L, H, E, M = 16, 1024, 8, 64, 64
BF = mybir.dt.bfloat16
F32 = mybir.dt.float32
NPBF = ml_dtypes.bfloat16

N_WARMUP = 24


def kernel_body(tc, outs, ins):
    nc = tc.nc
    ys = outs
    xq, fb, s, g = ins

    with (
        tc.tile_pool(name="const", bufs=1) as const,
        tc.tile_pool(name="work", bufs=1) as work,
        tc.tile_pool(name="yout", bufs=8) as yout,
        tc.tile_pool(name="phalf", bufs=3, space="PSUM") as phalf,
        tc.tile_pool(name="pyh", bufs=5, space="PSUM") as pyh,
    ):
        # ---- input DMA: all on the sync queue, priority-ordered ----
        fb_sb = const.tile([128, 8 * 128], BF, tag="fb")
        # head chunk first: the c=0 stationary (32KB) unblocks the first
        # fwd-DFT matmul ~2us before the rest of fb/xq lands.
        nc.sync.dma_start(fb_sb[:, 0:128], fb[:, 0:128])
        xq_sb = const.tile([128, 8 * 1024], BF, tag="xq")
        s_sb = const.tile([128, 64 * 128], BF, tag="s")
        def xq_dma_cols(lo, hi):
            nc.sync.dma_start(xq_sb[:, lo:hi], xq[:, lo:hi])
        def s_dma(c):
            nc.sync.dma_start(
                s_sb[:, c * 2048 : (c + 1) * 2048], s[:, c * 2048 : (c + 1) * 2048]
            )
        # leading chunks are small so the first fwd-DFT matmuls start while
        # the DMA rate is still ramping; trailing chunks are big for rate.
        xq_dma_cols(0, 1024)
        nc.sync.dma_start(fb_sb[:, 128:1024], fb[:, 128:1024])
        for lo, hi in ((1024, 2048), (2048, 3072), (3072, 4096),
                       (4096, 6144), (6144, 8192)):
            xq_dma_cols(lo, hi)
        for c in range(4):
            s_dma(c)
        g_sb = const.tile([128, 1024], BF, tag="g")
        nc.sync.dma_start(g_sb[:], g[:])

        ident = const.tile([128, 128], BF, tag="ident")
        make_identity(nc, ident[:])

        # ---- PE warmup: ramp the clock while input DMA streams ----
        warm = phalf.tile([128, 512], F32, tag="half")
        for _ in range(N_WARMUP):
            nc.tensor.matmul(warm[:, 0:128], ident[:], ident[:], start=True, stop=True)

        # ---- forward DFT: X[m_ri, (b,i)]; accumulate both halves per chunk ----
        Xp0 = phalf.tile([128, 512], F32, tag="half")
        Xp1 = phalf.tile([128, 512], F32, tag="half")
        for c in range(8):
            nc.tensor.matmul(
                Xp0[:],
                fb_sb[:, c * 128 : (c + 1) * 128],
                xq_sb[:, c * 1024 : c * 1024 + 512],
                start=(c == 0),
                stop=(c == 7),
            )
            nc.tensor.matmul(
                Xp1[:],
                fb_sb[:, c * 128 : (c + 1) * 128],
                xq_sb[:, c * 1024 + 512 : (c + 1) * 1024],
                start=(c == 0),
                stop=(c == 7),
            )
        X_sb = work.tile([128, 1024], BF, tag="xsb")
        nc.vector.tensor_copy(X_sb[:, 0:512], Xp0[:])
        nc.scalar.copy(X_sb[:, 512:1024], Xp1[:])

        # ---- stage A transposes: Mst[i_ri, (b,m)]  (free col = b*64 + m) ----
        # plain matmuls: out = lhsT.T @ I, grouped h0-then-h64 per PSUM tile.
        Mst = work.tile([128, 1024], BF, tag="mst")
        for half in (0, 1):
            Mp = phalf.tile([128, 512], F32, tag="half")
            bs = range(half * 8, half * 8 + 8)
            for b in bs:
                cols = slice(b * 64 - half * 512, (b + 1) * 64 - half * 512)
                icols = slice(b * 64, (b + 1) * 64)
                nc.tensor.matmul(
                    Mp[0:64, cols], X_sb[0:64, icols], ident[0:64, 0:64],
                    start=True, stop=True,
                )
            for b in bs:
                cols = slice(b * 64 - half * 512, (b + 1) * 64 - half * 512)
                icols = slice(b * 64, (b + 1) * 64)
                nc.tensor.matmul(
                    Mp[64:128, cols], X_sb[64:128, icols], ident[64:128, 64:128],
                    start=True, stop=True,
                )
            if half == 0:
                nc.vector.tensor_copy(Mst[:, 0:512], Mp[:])
            else:
                nc.scalar.copy(Mst[:, 512:1024], Mp[:])

        # ---- einsum: O[o_ri, (m,b)] -> stored b-major in O_sb ----
        O_sb = work.tile([128, 1024], BF, tag="osb")
        for half in (0, 1):
            Op = phalf.tile([128, 512], F32, tag="half")
            for m in range(half * 32, half * 32 + 32):
                nc.tensor.matmul(
                    Op[:, m * 16 - half * 512 : (m + 1) * 16 - half * 512],
                    s_sb[:, m * 128 : (m + 1) * 128],
                    Mst[:, m : 1024 : 64],
                    start=True,
                    stop=True,
                )
            for q in (0, 1):
                mm_lo = half * 32 + q * 16
                dst = O_sb.rearrange("p (b mm) -> p b mm", b=16)[
                    :, :, mm_lo : mm_lo + 16
                ]
                srcq = Op.rearrange("p (mm b) -> p b mm", b=16)[
                    :, :, q * 16 : (q + 1) * 16
                ]
                if half == 0 or q == 0:
                    nc.scalar.copy(dst, srcq)
                else:
                    nc.vector.tensor_copy(dst, srcq)

        # ---- stage B transposes: lhsT_all[m_ri, (k, j, o)] ----
        lhsT_all = work.tile([128, 1024], BF, tag="lhsT")
        for half in (0, 1):
            Pp = phalf.tile([128, 512], F32, tag="half")
            kjs = [(k, j) for k in range(half * 4, half * 4 + 4) for j in (0, 1)]
            for k, j in kjs:
                b = 2 * k + j
                cols = slice(
                    k * 128 + j * 64 - half * 512,
                    k * 128 + (j + 1) * 64 - half * 512,
                )
                bcols = slice(b * 64, (b + 1) * 64)
                nc.tensor.matmul(
                    Pp[0:64, cols], O_sb[0:64, bcols], ident[0:64, 0:64],
                    start=True, stop=True,
                )
            for k, j in kjs:
                b = 2 * k + j
                cols = slice(
                    k * 128 + j * 64 - half * 512,
                    k * 128 + (j + 1) * 64 - half * 512,
                )
                bcols = slice(b * 64, (b + 1) * 64)
                nc.tensor.matmul(
                    Pp[64:128, cols], O_sb[64:128, bcols], ident[64:128, 64:128],
                    start=True, stop=True,
                )
            if half == 0:
                nc.vector.tensor_copy(lhsT_all[:, 0:512], Pp[:])
            else:
                nc.scalar.copy(lhsT_all[:, 512:1024], Pp[:])

        # ---- inverse DFT per batch-pair; y DMA alternates sync/scalar queues ----
        for k in range(8):
            Yh0 = pyh.tile([128, 512], F32, tag="yh")
            Yh1 = pyh.tile([128, 512], F32, tag="yh")
            lk = lhsT_all[:, k * 128 : (k + 1) * 128]
            nc.tensor.matmul(Yh0[:], lk, g_sb[:, 0:512], start=True, stop=True)
            nc.tensor.matmul(Yh1[:], lk, g_sb[:, 512:1024], start=True, stop=True)
            y_sb = yout.tile([128, 1024], BF, tag="ysb")
            if k % 2 == 1:
                nc.vector.tensor_copy(y_sb[:, 0:512], Yh0[:])
                nc.vector.tensor_copy(y_sb[:, 512:1024], Yh1[:])
            else:
                nc.scalar.copy(y_sb[:, 0:512], Yh0[:])
                nc.scalar.copy(y_sb[:, 512:1024], Yh1[:])
            nc.sync.dma_start(ys[k][:], y_sb[:])


def build_nc():
    nc = bacc.Bacc(
        "TRN2", target_bir_lowering=False, debug=False, num_devices=8
    )
    xq = nc.dram_tensor("xq", [128, 8 * 1024], BF, kind="ExternalInput").ap()
    fb = nc.dram_tensor("fb", [128, 8 * 128], BF, kind="ExternalInput").ap()
    s = nc.dram_tensor("s", [128, 64 * 128], BF, kind="ExternalInput").ap()
    g = nc.dram_tensor("g", [128, 1024], BF, kind="ExternalInput").ap()
    ys = [
        nc.dram_tensor(f"y{k}", [128, 1024], BF, kind="ExternalOutput").ap()
        for k in range(8)
    ]
    with tile.TileContext(nc) as tc:
        kernel_body(tc, ys, [xq, fb, s, g])
    nc.compile()
    return nc


def host_basis():
    l = np.arange(L, dtype=np.float64)[:, None]
    m = np.arange(M, dtype=np.float64)[None, :]
    ang = 2 * np.pi * l * m / L
    FB = np.concatenate([np.cos(ang), -np.sin(ang)], axis=1)  # [L, 128]
    c = np.full(M, 2.0)
    c[0] = 1.0
    GC = c[:, None] * np.cos(ang.T) / L
    GS = -c[:, None] * np.sin(ang.T) / L
    G = np.concatenate([GC, GS], axis=0)  # [128, L]
    # chunk-major layout for direct [128, ...] DMA
    fb_host = np.ascontiguousarray(
        FB.reshape(8, 128, 128).transpose(1, 0, 2).reshape(128, 1024)
    ).astype(NPBF)
    g_host = np.ascontiguousarray(G).astype(NPBF)
    return fb_host, g_host


def host_inputs(q, w_real, w_imag):
    fb_host, g_host = host_basis()
    in_maps = []
    for h in range(H):
        x = q[:, :, h, :]  # [B, L, E]
        xT = np.transpose(x, (1, 0, 2)).reshape(L, B * E)  # [L, (b,i)] b-major
        xq_host = np.ascontiguousarray(
            xT.reshape(8, 128, B * E).transpose(1, 0, 2).reshape(128, 8 * 1024)
        ).astype(NPBF)
        # einsum stationaries: S_m = [[Wr, Wi], [-Wi, Wr]]  (rows i_ri, cols o_ri)
        Wr = w_real[h].astype(np.float32)  # [i, o, m]
        Wi = w_imag[h].astype(np.float32)
        Sm = np.empty((M, 128, 128), dtype=np.float32)
        Sm[:, 0:64, 0:64] = Wr.transpose(2, 0, 1)
        Sm[:, 0:64, 64:128] = Wi.transpose(2, 0, 1)
        Sm[:, 64:128, 0:64] = -Wi.transpose(2, 0, 1)
        Sm[:, 64:128, 64:128] = Wr.transpose(2, 0, 1)
        s_host = np.ascontiguousarray(Sm.transpose(1, 0, 2).reshape(128, 8192)).astype(
            NPBF
        )
        in_maps.append({"xq": xq_host, "fb": fb_host, "s": s_host, "g": g_host})
    return in_maps


def assemble(results):
    out = np.empty((B, H, E, L), dtype=np.float32)
    for h in range(H):
        yh = np.stack(
            [results[h][f"y{k}"].astype(np.float32) for k in range(8)]
        )  # [k, 128, L]
        out[:, h, :, :] = yh.reshape(B, E, L)  # [(k,j)=b, o, l]
    return out


_NC_CACHE = {}


def run(q, w_real, w_imag, **kwargs):
    if "nc" not in _NC_CACHE:
        _NC_CACHE["nc"] = build_nc()
    nc = _NC_CACHE["nc"]
    in_maps = host_inputs(
        np.asarray(q, dtype=np.float32),
        np.asarray(w_real, dtype=np.float32),
        np.asarray(w_imag, dtype=np.float32),
    )
    res = run_bass_kernel_spmd(nc, in_maps, core_ids=list(range(H)), **kwargs)
    return assemble(res.results), res


def kernel(q, w_real, w_imag):
    return run(q, w_real, w_imag)[0]
